# revision 5
# baseline (speedup 1.0000x reference)
"""Trainium2 Bass kernel for the CapacityNN PINN forward pass.

Computes, for N = B*S collocation points x = (s, t):
  U   = MLP([s_norm, t_norm]) * tgt_std + tgt_mean
  F   = U_t  - G(U)             (G = Verhulst logistic growth term)
  F_t = U_tt - G'(U) * U_t
where U_t/U_tt are 1st/2nd derivatives w.r.t. t_norm, computed exactly by
forward-mode Taylor (jet) propagation through the tanh MLP.

Sharding: pure data parallel over 8 NeuronCores (8192 points/core),
MLP weights + PDE scalars replicated. All math runs on-device; the host
only reorders data (transposes for layout, shard/gather).

Device layout: hidden dim (256) on partitions (2 tiles of 128), points on
the free dim, processed in chunks of 2048 points. Streams:
  Hv = values (fp32), H1 = sigma*dU/dt_norm, H2 = sigma*d2U/dt_norm2
with a compile-time sign convention sigma that flips each hidden layer
(because (e-1) = -(1-tanh^2) is a single fused DVE op), fixed up in the
final projection scale.
"""

import os
import sys
import tempfile

import numpy as np

for _p in ("/opt/trn_rl_repo", "/root/.axon_site/_ro/trn_rl_repo"):
    if os.path.isdir(_p) and _p not in sys.path:
        sys.path.insert(0, _p)

import concourse.bass as bass
import concourse.bacc as bacc
import concourse.tile as tile
from concourse import mybir
from concourse.bass_utils import run_bass_kernel_spmd

AF = mybir.ActivationFunctionType
OP = mybir.AluOpType
F32 = mybir.dt.float32

NCORES = 8
B, S, H = 512, 128, 256
N = B * S                  # 65536 points
NLOC = N // NCORES         # 8192 points per core
CH = 2048                  # points per on-chip chunk
NCHUNK = NLOC // CH
PT = CH // 512             # 512-wide matmul point tiles per chunk
CHP = CH // 128            # points per partition in the tail layout
SQRT2 = float(np.sqrt(2.0))


def _build():
    nc = bacc.Bacc(
        "TRN2",
        target_bir_lowering=False,
        debug=False,
        enable_asserts=False,
        num_devices=NCORES,
    )

    def din(name, shape):
        return nc.dram_tensor(name, list(shape), F32, kind="ExternalInput").ap()

    x2 = din("x2", (2, NLOC))            # rows: raw s, raw t (per-core slice)
    w0t = din("w0t", (2, H))             # W0.T
    wts = {l: din(f"w{l}t", (H, H)) for l in (1, 2, 3)}   # Wl.T
    w4 = din("w4", (1, H))
    bs = {l: din(f"b{l}", (H,)) for l in range(4)}
    b4 = din("b4", (1,))
    lgr = din("lgr", (1,))
    lcc = din("lcc", (1,))
    lil = din("lil", (1,))
    in_mean = din("in_mean", (2,))
    in_std = din("in_std", (2,))
    tgt_mean = din("tgt_mean", (1,))
    tgt_std = din("tgt_std", (1,))
    out = nc.dram_tensor("out", [3, NLOC], F32, kind="ExternalOutput").ap()

    with tile.TileContext(nc) as tc:
        from contextlib import ExitStack

        with ExitStack() as ctx:
            const = ctx.enter_context(tc.tile_pool(name="const", bufs=1))
            sb = ctx.enter_context(tc.tile_pool(name="sb", bufs=1))
            ps = ctx.enter_context(tc.tile_pool(name="ps", bufs=1, space="PSUM"))

            # ---------- one-time prep: broadcast scalars to [128,1] ----------
            def bc_tile(src_ap, off, name):
                t = const.tile([128, 1], F32, name=name)
                nc.sync.dma_start(
                    out=t, in_=bass.AP(src_ap.tensor, off, [[0, 128], [1, 1]])
                )
                return t

            bc_m0 = bc_tile(in_mean, 0, "bc_m0")
            bc_m1 = bc_tile(in_mean, 1, "bc_m1")
            bc_s0 = bc_tile(in_std, 0, "bc_s0")
            bc_s1 = bc_tile(in_std, 1, "bc_s1")
            bc_lgr = bc_tile(lgr, 0, "bc_lgr")
            bc_lcc = bc_tile(lcc, 0, "bc_lcc")
            bc_lil = bc_tile(lil, 0, "bc_lil")
            bc_tm = bc_tile(tgt_mean, 0, "bc_tm")
            bc_ts = bc_tile(tgt_std, 0, "bc_ts")
            bc_b4 = bc_tile(b4, 0, "bc_b4")

            def new1(name):
                return const.tile([128, 1], F32, name=name)

            inv0 = new1("inv0")
            nc.vector.tensor_scalar(inv0, bc_s0, 1e-8, None, OP.add)
            nc.vector.reciprocal(inv0, inv0)
            inv1 = new1("inv1")
            nc.vector.tensor_scalar(inv1, bc_s1, 1e-8, None, OP.add)
            nc.vector.reciprocal(inv1, inv1)

            r_t = new1("r_t")
            nc.scalar.activation(r_t, bc_lgr, AF.Exp, 0.0, -1.0)   # exp(-lgr)
            K_t = new1("K_t")
            nc.scalar.activation(K_t, bc_lcc, AF.Sigmoid)
            nc.vector.tensor_scalar(K_t, K_t, 0.8, 0.2, OP.mult, OP.add)
            C_t = new1("C_t")
            nc.scalar.activation(C_t, bc_lil, AF.Sigmoid)
            nc.vector.tensor_scalar(C_t, C_t, 0.1, None, OP.mult)
            ikc = new1("ikc")                                      # 1/(K-C)
            nc.vector.tensor_tensor(ikc, K_t, C_t, OP.subtract)
            nc.vector.reciprocal(ikc, ikc)
            nr = new1("nr")                                        # -r
            nc.vector.tensor_scalar(nr, r_t, -1.0, None, OP.mult)
            c1 = new1("c1")                                        # -1/(K-C)
            nc.vector.tensor_scalar(c1, ikc, -1.0, None, OP.mult)
            mc3 = new1("mc3")                                      # 2r/(K-C)
            nc.vector.tensor_tensor(mc3, r_t, ikc, OP.mult)
            nc.vector.tensor_scalar(mc3, mc3, 2.0, None, OP.mult)
            sts = new1("sts")                                      # -tgt_std (sigma3)
            nc.vector.tensor_scalar(sts, bc_ts, -1.0, None, OP.mult)
            tmb = new1("tmb")                                      # b4*ts + tm
            nc.vector.tensor_tensor(tmb, bc_b4, bc_ts, OP.mult)
            nc.vector.tensor_tensor(tmb, tmb, bc_tm, OP.add)
            m0i = new1("m0i")                                      # m0/(s0+eps)
            nc.vector.tensor_tensor(m0i, bc_m0, inv0, OP.mult)
            m1i = new1("m1i")
            nc.vector.tensor_tensor(m1i, bc_m1, inv1, OP.mult)

            # ---------- layer-0 folded weights ----------
            inv01 = const.tile([2, 1], F32, name="inv01")
            nc.sync.dma_start(
                out=inv01, in_=bass.AP(in_std.tensor, 0, [[1, 2], [1, 1]])
            )
            nc.vector.tensor_scalar(inv01, inv01, 1e-8, None, OP.add)
            nc.vector.reciprocal(inv01, inv01)
            w0ts = const.tile([2, H], F32, name="w0ts")            # rows scaled by 1/std
            nc.sync.dma_start(out=w0ts, in_=w0t)
            nc.vector.tensor_scalar(w0ts, w0ts, inv01, None, OP.mult)

            nw, mw, beta0 = [], [], []
            for m in range(2):
                a = const.tile([128, 2], F32, name=f"w0c_{m}")     # W0 rows [128m:128m+128]
                nc.sync.dma_start(
                    out=a, in_=bass.AP(w0t.tensor, 128 * m, [[1, 128], [H, 2]])
                )
                nwm = new1(f"nw_{m}")                              # -W0[:,1]
                nc.vector.tensor_scalar(nwm, a[:, 1:2], -1.0, None, OP.mult)
                mwm = new1(f"mw_{m}")                              # -2*W0[:,1]
                nc.vector.tensor_scalar(mwm, a[:, 1:2], -2.0, None, OP.mult)
                u1 = new1(f"u1_{m}")
                nc.vector.tensor_tensor(u1, a[:, 0:1], m0i, OP.mult)
                u2 = new1(f"u2_{m}")
                nc.vector.tensor_tensor(u2, a[:, 1:2], m1i, OP.mult)
                nc.vector.tensor_tensor(u1, u1, u2, OP.add)
                bb = new1(f"bb0_{m}")
                nc.sync.dma_start(
                    out=bb, in_=bass.AP(bs[0].tensor, 128 * m, [[1, 128], [1, 1]])
                )
                bet = new1(f"beta_{m}")                            # b0 - u1
                nc.vector.scalar_tensor_tensor(bet, u1, -1.0, bb, OP.mult, OP.add)
                nw.append(nwm)
                mw.append(mwm)
                beta0.append(bet)

            # ---------- hidden-layer weights (pre-transposed on host) ----------
            wt = {l: [[None] * 2 for _ in range(2)] for l in (1, 2, 3)}
            for l in (1, 2, 3):
                for kk in range(2):
                    for mm in range(2):
                        t = const.tile([128, 128], F32, name=f"wt{l}_{kk}{mm}")
                        nc.sync.dma_start(
                            out=t,
                            in_=bass.AP(
                                wts[l].tensor,
                                kk * 128 * H + mm * 128,
                                [[H, 128], [1, 128]],
                            ),
                        )
                        wt[l][kk][mm] = t

            bl = {}
            for l in (1, 2, 3):
                bl[l] = []
                for m in range(2):
                    t = new1(f"bl{l}_{m}")
                    nc.sync.dma_start(
                        out=t, in_=bass.AP(bs[l].tensor, 128 * m, [[1, 128], [1, 1]])
                    )
                    bl[l].append(t)

            # final-projection block-diagonal lhsT tiles: [128,3], col s = W4 half
            lt4 = [[None] * 2 for _ in range(3)]
            for s_idx in range(3):
                for kk in range(2):
                    t = const.tile([128, 3], F32, name=f"lt4_{s_idx}{kk}")
                    nc.vector.memset(t, 0.0)
                    nc.sync.dma_start(
                        out=t[:, s_idx : s_idx + 1],
                        in_=bass.AP(w4.tensor, kk * 128, [[1, 128], [1, 1]]),
                    )
                    lt4[s_idx][kk] = t

            # ---------- main loop over point chunks ----------
            sigma_in = {1: 1.0, 2: -1.0, 3: 1.0}
            for c in range(NCHUNK):
                x2c = sb.tile([2, CH], F32, tag="x2c", bufs=2)
                nc.sync.dma_start(out=x2c, in_=x2[:, c * CH : (c + 1) * CH])

                Hv = [None] * 2
                H1 = [None] * 2
                H2 = [None] * 2

                # ----- layer 0 (K=2 matmul; derivative streams are cheap) -----
                for m in range(2):
                    pz = ps.tile([128, CH], F32, tag="pz", bufs=2)
                    for i in range(PT):
                        nc.tensor.matmul(
                            pz[:, i * 512 : (i + 1) * 512],
                            w0ts[:, m * 128 : (m + 1) * 128],
                            x2c[:, i * 512 : (i + 1) * 512],
                            start=True,
                            stop=True,
                        )
                    av = sb.tile([128, CH], F32, tag=f"hv{m}", bufs=2)
                    nc.scalar.activation(av, pz, AF.Tanh, beta0[m])
                    ee = sb.tile([128, CH], F32, tag=f"ee{m}", bufs=1)
                    nc.scalar.activation(ee, av, AF.Square)
                    h1t = sb.tile([128, CH], F32, tag=f"h1{m}", bufs=2)
                    # (e-1)*(-w) = (1-e)*w = d*z0'
                    nc.vector.tensor_scalar(h1t, ee, 1.0, nw[m], OP.subtract, OP.mult)
                    h2t = sb.tile([128, CH], F32, tag=f"h2{m}", bufs=2)
                    # (a*-2w)*h1 = -2*w^2*a*d
                    nc.vector.scalar_tensor_tensor(
                        h2t, av, mw[m], h1t, OP.mult, OP.mult
                    )
                    Hv[m], H1[m], H2[m] = av, h1t, h2t

                # ----- hidden layers 1..3 -----
                for l in (1, 2, 3):
                    s_in = sigma_in[l]
                    nHv = [None] * 2
                    nEe = [None] * 2
                    nH1 = [None] * 2
                    nH2 = [None] * 2
                    St = [None] * 2
                    Tt = [None] * 2
                    # primal
                    for m in range(2):
                        pz = ps.tile([128, CH], F32, tag="pz", bufs=2)
                        for i in range(PT):
                            for kk in range(2):
                                nc.tensor.matmul(
                                    pz[:, i * 512 : (i + 1) * 512],
                                    wt[l][kk][m],
                                    Hv[kk][:, i * 512 : (i + 1) * 512],
                                    start=(kk == 0),
                                    stop=(kk == 1),
                                )
                        av = sb.tile([128, CH], F32, tag=f"hv{m}", bufs=2)
                        nc.scalar.activation(av, pz, AF.Tanh, bl[l][m])
                        ee = sb.tile([128, CH], F32, tag=f"ee{m}", bufs=1)
                        nc.scalar.activation(ee, av, AF.Square)
                        nHv[m], nEe[m] = av, ee
                    # first-derivative stream
                    for m in range(2):
                        pz1 = ps.tile([128, CH], F32, tag="pz", bufs=2)
                        for i in range(PT):
                            for kk in range(2):
                                nc.tensor.matmul(
                                    pz1[:, i * 512 : (i + 1) * 512],
                                    wt[l][kk][m],
                                    H1[kk][:, i * 512 : (i + 1) * 512],
                                    start=(kk == 0),
                                    stop=(kk == 1),
                                )
                        st = sb.tile([128, CH], F32, tag=f"st{m}", bufs=1)
                        nc.scalar.activation(st, pz1, AF.Square, 0.0, SQRT2)  # 2*z'^2
                        h1t = sb.tile([128, CH], F32, tag=f"h1{m}", bufs=2)
                        nc.vector.scalar_tensor_tensor(
                            h1t, nEe[m], 1.0, pz1, OP.subtract, OP.mult
                        )  # (e-1)*Z1
                        tt = sb.tile([128, CH], F32, tag=f"tt{m}", bufs=1)
                        nc.vector.tensor_tensor(tt, nHv[m], st, OP.mult)  # a*s
                        nH1[m], St[m], Tt[m] = h1t, st, tt
                    # second-derivative stream
                    for m in range(2):
                        pz2 = ps.tile([128, CH], F32, tag="pz", bufs=2)
                        for i in range(PT):
                            for kk in range(2):
                                nc.tensor.matmul(
                                    pz2[:, i * 512 : (i + 1) * 512],
                                    wt[l][kk][m],
                                    H2[kk][:, i * 512 : (i + 1) * 512],
                                    start=(kk == 0),
                                    stop=(kk == 1),
                                )
                        qt = sb.tile([128, CH], F32, tag=f"qt{m}", bufs=1)
                        nc.vector.scalar_tensor_tensor(
                            qt, Tt[m], -s_in, pz2, OP.mult, OP.add
                        )  # sigma*(z''-t)
                        h2t = sb.tile([128, CH], F32, tag=f"h2{m}", bufs=2)
                        nc.vector.scalar_tensor_tensor(
                            h2t, nEe[m], 1.0, qt, OP.subtract, OP.mult
                        )  # (e-1)*q
                        nH2[m] = h2t
                    Hv, H1, H2 = nHv, nH1, nH2

                # ----- final projection: block-diag lhsT -> psum rows (y,y',y'') -----
                y3 = sb.tile([3, CH], F32, tag="y3", bufs=2)
                for i in range(PT):
                    py = ps.tile([3, 512], F32, tag="pz", bufs=2)
                    first = True
                    for s_idx, stream in enumerate((Hv, H1, H2)):
                        for kk in range(2):
                            nc.tensor.matmul(
                                py,
                                lt4[s_idx][kk],
                                stream[kk][:, i * 512 : (i + 1) * 512],
                                start=first,
                                stop=(s_idx == 2 and kk == 1),
                            )
                            first = False
                    nc.scalar.copy(y3[:, i * 512 : (i + 1) * 512], py)

                # ----- tail: reshape to [128, CHP] per stream, PDE algebra -----
                tp = sb.tile([128, 3 * CHP], F32, tag="tp", bufs=2)
                for s_idx in range(3):
                    nc.sync.dma_start(
                        out=tp[:, s_idx * CHP : (s_idx + 1) * CHP],
                        in_=y3[s_idx : s_idx + 1, :],
                    )
                yv = tp[:, 0:CHP]
                yt = tp[:, CHP : 2 * CHP]
                ytt = tp[:, 2 * CHP : 3 * CHP]
                oc = sb.tile([128, 3 * CHP], F32, tag="oc", bufs=2)
                U = oc[:, 0:CHP]
                Fo = oc[:, CHP : 2 * CHP]
                Ft = oc[:, 2 * CHP : 3 * CHP]

                def tl(name):
                    return sb.tile([128, CHP], F32, tag=name, bufs=2, name=name)

                ut, utt, vv, v2, w1, q1, t1 = (
                    tl("ut"), tl("utt"), tl("vv"), tl("v2"), tl("w1"), tl("q1"), tl("t1"),
                )
                nc.vector.tensor_scalar(U, yv, bc_ts, tmb, OP.mult, OP.add)
                nc.vector.tensor_scalar(ut, yt, sts, None, OP.mult)
                nc.vector.tensor_scalar(utt, ytt, sts, None, OP.mult)
                nc.vector.tensor_scalar(vv, U, C_t, None, OP.subtract)
                nc.vector.tensor_tensor(v2, vv, vv, OP.mult)
                nc.vector.scalar_tensor_tensor(w1, v2, c1, vv, OP.mult, OP.add)
                nc.vector.scalar_tensor_tensor(Fo, w1, nr, ut, OP.mult, OP.add)
                nc.vector.tensor_tensor(q1, vv, ut, OP.mult)
                nc.vector.scalar_tensor_tensor(t1, ut, nr, utt, OP.mult, OP.add)
                nc.vector.scalar_tensor_tensor(Ft, q1, mc3, t1, OP.mult, OP.add)
                for s_idx, src in enumerate((U, Fo, Ft)):
                    nc.sync.dma_start(
                        out=out[s_idx : s_idx + 1, c * CH : (c + 1) * CH], in_=src
                    )

    nc.compile()
    return nc


_STATE = {}


def _get_nc():
    if "nc" not in _STATE:
        _STATE["nc"] = _build()
    return _STATE["nc"]


def _prep_in_maps(inputs):
    f = np.float32

    def arr(k):
        return np.ascontiguousarray(np.asarray(inputs[k], f))

    x = np.asarray(inputs["inputs"], f).reshape(N, 2)
    shared = {
        "w0t": np.ascontiguousarray(arr("W0").T),
        "w1t": np.ascontiguousarray(arr("W1").T),
        "w2t": np.ascontiguousarray(arr("W2").T),
        "w3t": np.ascontiguousarray(arr("W3").T),
        "w4": arr("W4").reshape(1, H),
        "b0": arr("b0"),
        "b1": arr("b1"),
        "b2": arr("b2"),
        "b3": arr("b3"),
        "b4": arr("b4").reshape(1),
        "lgr": arr("log_growth_rate").reshape(1),
        "lcc": arr("log_carrying_capacity").reshape(1),
        "lil": arr("log_initial_loss").reshape(1),
        "in_mean": arr("in_mean"),
        "in_std": arr("in_std"),
        "tgt_mean": arr("tgt_mean"),
        "tgt_std": arr("tgt_std"),
    }
    in_maps = []
    for c in range(NCORES):
        m = dict(shared)
        m["x2"] = np.ascontiguousarray(x[c * NLOC : (c + 1) * NLOC].T)
        in_maps.append(m)
    return in_maps


def run(inputs, trace=False):
    nc = _get_nc()
    in_maps = _prep_in_maps(inputs)
    kw = {}
    if trace:
        kw["tmpdir"] = tempfile.mkdtemp(prefix="bassk_prof_")
    res = run_bass_kernel_spmd(
        nc, in_maps, core_ids=list(range(NCORES)), trace=trace, **kw
    )
    U = np.empty((N,), np.float32)
    F = np.empty((N,), np.float32)
    Ft = np.empty((N,), np.float32)
    for c in range(NCORES):
        o = res.results[c]["out"]
        U[c * NLOC : (c + 1) * NLOC] = o[0]
        F[c * NLOC : (c + 1) * NLOC] = o[1]
        Ft[c * NLOC : (c + 1) * NLOC] = o[2]
    shp = (B, S, 1)
    return (U.reshape(shp), F.reshape(shp), Ft.reshape(shp)), res


def kernel(**inputs):
    outs, _ = run(inputs, trace=False)
    return outs


# ---------------------------------------------------------------------------
# Dev-loop timing: persistent jitted executable (mirrors
# bass2jax.run_bass_via_pjrt's multi-core branch) so repeated executions
# reuse one compiled NEFF and can be timed back-to-back.
# ---------------------------------------------------------------------------
def _make_runner():
    if "runner" in _STATE:
        return _STATE["runner"]
    import jax
    from jax.experimental.shard_map import shard_map
    from jax.sharding import Mesh, PartitionSpec
    from concourse import bass2jax

    bass2jax.install_neuronx_cc_hook()
    nc = _get_nc()

    in_names, out_names, out_avals, zero_outs = [], [], [], []
    for alloc in nc.m.functions[0].allocations:
        if not isinstance(alloc, mybir.MemoryLocationSet):
            continue
        name = alloc.memorylocations[0].name
        if alloc.kind == "ExternalInput":
            if nc.partition_id_tensor is None or name != nc.partition_id_tensor.name:
                in_names.append(name)
        elif alloc.kind == "ExternalOutput":
            out_names.append(name)
            shape = tuple(alloc.tensor_shape)
            dtype = mybir.dt.np(alloc.dtype)
            out_avals.append(jax.core.ShapedArray(shape, dtype))
            zero_outs.append(np.zeros(shape, dtype))
    n_params = len(in_names)
    n_outs = len(out_avals)
    all_names = in_names + out_names
    if nc.partition_id_tensor is not None:
        all_names = all_names + [nc.partition_id_tensor.name]

    def _body(*args):
        operands = list(args)
        if nc.partition_id_tensor is not None:
            operands.append(bass2jax.partition_id_tensor())
        outs = bass2jax._bass_exec_p.bind(
            *operands,
            out_avals=tuple(out_avals),
            in_names=tuple(all_names),
            out_names=tuple(out_names),
            lowering_input_output_aliases=(),
            sim_require_finite=True,
            sim_require_nnan=True,
            nc=nc,
        )
        return tuple(outs)

    devices = jax.devices()[:NCORES]
    mesh = Mesh(np.asarray(devices), ("core",))
    donate = tuple(range(n_params, n_params + n_outs))
    sharded = jax.jit(
        shard_map(
            _body,
            mesh=mesh,
            in_specs=(PartitionSpec("core"),) * (n_params + n_outs),
            out_specs=(PartitionSpec("core"),) * n_outs,
            check_rep=False,
        ),
        donate_argnums=donate,
        keep_unused=True,
    )
    _STATE["runner"] = (sharded, in_names, out_names, out_avals, zero_outs)
    return _STATE["runner"]


def run_timed(inputs, iters=20):
    """Run via a persistent executable; return (outputs, per_iter_ns)."""
    import time as _time

    import jax

    sharded, in_names, out_names, out_avals, zero_outs = _make_runner()
    in_maps = _prep_in_maps(inputs)
    concat_in = [
        np.concatenate([np.asarray(in_maps[c][n]) for c in range(NCORES)], axis=0)
        for n in in_names
    ]
    dev_in = [jax.device_put(a) for a in concat_in]

    def zeros():
        return [
            np.zeros((NCORES * z.shape[0], *z.shape[1:]), z.dtype) for z in zero_outs
        ]

    # warmup (compiles on first call)
    outs = sharded(*dev_in, *zeros())
    jax.block_until_ready(outs)
    out_np = [np.asarray(o) for o in outs]

    zbufs = [zeros() for _ in range(iters)]
    t0 = _time.perf_counter()
    last = None
    for i in range(iters):
        last = sharded(*dev_in, *zbufs[i])
    jax.block_until_ready(last)
    t1 = _time.perf_counter()
    per_iter_ns = (t1 - t0) / iters * 1e9

    per_core = [
        {
            name: out_np[i].reshape(NCORES, *out_avals[i].shape)[c]
            for i, name in enumerate(out_names)
        }
        for c in range(NCORES)
    ]
    U = np.empty((N,), np.float32)
    F = np.empty((N,), np.float32)
    Ft = np.empty((N,), np.float32)
    for c in range(NCORES):
        o = per_core[c]["out"]
        U[c * NLOC : (c + 1) * NLOC] = o[0]
        F[c * NLOC : (c + 1) * NLOC] = o[1]
        Ft[c * NLOC : (c + 1) * NLOC] = o[2]
    shp = (B, S, 1)
    return (U.reshape(shp), F.reshape(shp), Ft.reshape(shp)), per_iter_ns


# revision 31
# speedup vs baseline: 205.6822x; 205.6822x over previous
"""Trainium2 Bass kernel for the CapacityNN PINN forward pass.

Computes, for N = B*S collocation points x = (s, t):
  U   = MLP([s_norm, t_norm]) * tgt_std + tgt_mean
  F   = U_t  - G(U)             (G = Verhulst logistic growth term)
  F_t = U_tt - G'(U) * U_t
where U_t/U_tt are 1st/2nd derivatives w.r.t. t_norm, computed exactly by
forward-mode Taylor (jet) propagation through the tanh MLP.

Sharding: pure data parallel over 8 NeuronCores (8192 points/core),
MLP weights + PDE scalars replicated. All math runs on-device; the host
only reorders data (transposes for layout, shard/gather).

Device layout: hidden dim (256) on partitions (2 tiles of 128), points on
the free dim, processed in chunks of 2048 points. Streams:
  Hv = values (fp32), H1 = sigma*dU/dt_norm, H2 = sigma*d2U/dt_norm2
with a compile-time sign convention sigma that flips each hidden layer
(because (e-1) = -(1-tanh^2) is a single fused DVE op), fixed up in the
final projection scale.
"""

import os
import sys
import tempfile

import numpy as np

for _p in ("/opt/trn_rl_repo", "/root/.axon_site/_ro/trn_rl_repo"):
    if os.path.isdir(_p) and _p not in sys.path:
        sys.path.insert(0, _p)

import concourse.bass as bass
import concourse.bacc as bacc
import concourse.tile as tile
from concourse import mybir
from concourse.bass_utils import run_bass_kernel_spmd

AF = mybir.ActivationFunctionType
OP = mybir.AluOpType
F32 = mybir.dt.float32
F32R = mybir.dt.float32r
F16 = mybir.dt.float16

NCORES = 8
B, S, H = 512, 128, 256
N = B * S                  # 65536 points
NLOC = N // NCORES         # 8192 points per core
CH = 1024                  # points per on-chip chunk
NCHUNK = NLOC // CH
PT = CH // 512             # 512-wide matmul point tiles per chunk
PPP = NLOC // 128          # points per partition in the tail layout (64)
PG = 512                  # points per PSUM group (1 bank)
NG = CH // PG
SQRT2 = float(np.sqrt(2.0))


def _build():
    nc = bacc.Bacc(
        "TRN2",
        target_bir_lowering=False,
        debug=False,
        enable_asserts=False,
        num_devices=NCORES,
    )

    def din(name, shape, dt=F32):
        return nc.dram_tensor(name, list(shape), dt, kind="ExternalInput").ap()

    x2 = din("x2", (2, NLOC), F32R)            # rows: raw s, raw t (per-core slice)
    w0t = din("w0t", (2, H), F32R)             # W0.T
    wts = {l: din(f"w{l}t", (H, H), F32R) for l in (1, 2, 3)}   # Wl.T
    lt4d = din("lt4", (6, 128, 3), F32R)   # host-prepared block-diag final lhsT
    lt4h = din("lt4h", (6, 128, 3), F16)   # fp16 copy (derivative streams)
    wth = {l: din(f"w{l}th", (H, H), F16) for l in (1, 2, 3)}  # fp16 Wl.T
    w1wt = din("w1wt", (H, H), F16)     # (W1*diag(w0c1)).T fp16
    w1w2t = din("w1w2t", (H, H), F16)   # (W1*diag(-2*w0c1^2)).T fp16
    negid = din("negid", (128, 128), F16)  # -I for psum-accumulated subtraction
    bs = {l: din(f"b{l}", (H,)) for l in range(4)}
    b4 = din("b4", (1,))
    lgr = din("lgr", (1,))
    lcc = din("lcc", (1,))
    lil = din("lil", (1,))
    in_mean = din("in_mean", (2,))
    in_std = din("in_std", (2,))
    tgt_mean = din("tgt_mean", (1,))
    tgt_std = din("tgt_std", (1,))
    out = nc.dram_tensor("out", [3, NLOC], F32, kind="ExternalOutput").ap()

    with tile.TileContext(nc) as tc:
        from contextlib import ExitStack

        with ExitStack() as ctx:
            const = ctx.enter_context(tc.tile_pool(name="const", bufs=1))
            sb = ctx.enter_context(tc.tile_pool(name="sb", bufs=1))
            ps = ctx.enter_context(tc.tile_pool(name="ps", bufs=1, space="PSUM"))

            # ---------- one-time prep: broadcast scalars to [128,1] ----------
            def bc_tile(src_ap, off, name):
                t = const.tile([128, 1], F32, name=name)
                nc.sync.dma_start(
                    out=t, in_=bass.AP(src_ap.tensor, off, [[0, 128], [1, 1]])
                )
                return t

            bc_m0 = bc_tile(in_mean, 0, "bc_m0")
            bc_m1 = bc_tile(in_mean, 1, "bc_m1")
            bc_s0 = bc_tile(in_std, 0, "bc_s0")
            bc_s1 = bc_tile(in_std, 1, "bc_s1")
            bc_lgr = bc_tile(lgr, 0, "bc_lgr")
            bc_lcc = bc_tile(lcc, 0, "bc_lcc")
            bc_lil = bc_tile(lil, 0, "bc_lil")
            bc_tm = bc_tile(tgt_mean, 0, "bc_tm")
            bc_ts = bc_tile(tgt_std, 0, "bc_ts")
            bc_b4 = bc_tile(b4, 0, "bc_b4")

            def new1(name):
                return const.tile([128, 1], F32, name=name)

            inv0 = new1("inv0")
            nc.vector.tensor_scalar(inv0, bc_s0, 1e-8, None, OP.add)
            nc.vector.reciprocal(inv0, inv0)
            inv1 = new1("inv1")
            nc.vector.tensor_scalar(inv1, bc_s1, 1e-8, None, OP.add)
            nc.vector.reciprocal(inv1, inv1)

            r_t = new1("r_t")
            nc.scalar.activation(r_t, bc_lgr, AF.Exp, 0.0, -1.0)   # exp(-lgr)
            K_t = new1("K_t")
            nc.scalar.activation(K_t, bc_lcc, AF.Sigmoid)
            nc.vector.tensor_scalar(K_t, K_t, 0.8, 0.2, OP.mult, OP.add)
            C_t = new1("C_t")
            nc.scalar.activation(C_t, bc_lil, AF.Sigmoid)
            nc.vector.tensor_scalar(C_t, C_t, 0.1, None, OP.mult)
            ikc = new1("ikc")                                      # 1/(K-C)
            nc.vector.tensor_tensor(ikc, K_t, C_t, OP.subtract)
            nc.vector.reciprocal(ikc, ikc)
            nr = new1("nr")                                        # -r
            nc.vector.tensor_scalar(nr, r_t, -1.0, None, OP.mult)
            c1 = new1("c1")                                        # -1/(K-C)
            nc.vector.tensor_scalar(c1, ikc, -1.0, None, OP.mult)
            mc3 = new1("mc3")                                      # 2r/(K-C)
            nc.vector.tensor_tensor(mc3, r_t, ikc, OP.mult)
            nc.vector.tensor_scalar(mc3, mc3, 2.0, None, OP.mult)
            sts = bc_ts                                            # streams carry true sign
            tmb = new1("tmb")                                      # b4*ts + tm
            nc.vector.tensor_tensor(tmb, bc_b4, bc_ts, OP.mult)
            nc.vector.tensor_tensor(tmb, tmb, bc_tm, OP.add)
            m0i = new1("m0i")                                      # m0/(s0+eps)
            nc.vector.tensor_tensor(m0i, bc_m0, inv0, OP.mult)
            m1i = new1("m1i")
            nc.vector.tensor_tensor(m1i, bc_m1, inv1, OP.mult)

            # ---------- layer-0 folded weights ----------
            inv01 = const.tile([2, 1], F32, name="inv01")
            nc.sync.dma_start(
                out=inv01, in_=bass.AP(in_std.tensor, 0, [[1, 2], [1, 1]])
            )
            nc.vector.tensor_scalar(inv01, inv01, 1e-8, None, OP.add)
            nc.vector.reciprocal(inv01, inv01)
            w0ts = const.tile([2, H], F32R, name="w0ts")            # rows scaled by 1/std
            nc.sync.dma_start(out=w0ts, in_=w0t)
            nc.vector.tensor_scalar(w0ts, w0ts, inv01, None, OP.mult)

            beta0 = []
            for m in range(2):
                a = const.tile([128, 2], F32R, name=f"w0c_{m}")     # W0 rows [128m:128m+128]
                nc.sync.dma_start(
                    out=a, in_=bass.AP(w0t.tensor, 128 * m, [[1, 128], [H, 2]])
                )
                u1 = new1(f"u1_{m}")
                nc.vector.tensor_tensor(u1, a[:, 0:1], m0i, OP.mult)
                u2 = new1(f"u2_{m}")
                nc.vector.tensor_tensor(u2, a[:, 1:2], m1i, OP.mult)
                nc.vector.tensor_tensor(u1, u1, u2, OP.add)
                bb = new1(f"bb0_{m}")
                nc.sync.dma_start(
                    out=bb, in_=bass.AP(bs[0].tensor, 128 * m, [[1, 128], [1, 1]])
                )
                bet = new1(f"beta_{m}")                            # b0 - u1
                nc.vector.scalar_tensor_tensor(bet, u1, -1.0, bb, OP.mult, OP.add)
                beta0.append(bet)

            # ---------- hidden-layer weights (pre-transposed on host) ----------
            wt = {l: [[None] * 2 for _ in range(2)] for l in (1, 2, 3)}
            wt16 = {l: [[None] * 2 for _ in range(2)] for l in (1, 2, 3)}
            for l in (1, 2, 3):
                for kk in range(2):
                    for mm in range(2):
                        t = const.tile([128, 128], F32R, name=f"wt{l}_{kk}{mm}")
                        nc.sync.dma_start(
                            out=t,
                            in_=bass.AP(
                                wts[l].tensor,
                                kk * 128 * H + mm * 128,
                                [[H, 128], [1, 128]],
                            ),
                        )
                        wt[l][kk][mm] = t
                        th = const.tile([128, 128], F16, name=f"wth{l}_{kk}{mm}")
                        nc.sync.dma_start(
                            out=th,
                            in_=bass.AP(
                                wth[l].tensor,
                                kk * 128 * H + mm * 128,
                                [[H, 128], [1, 128]],
                            ),
                        )
                        wt16[l][kk][mm] = th
            wtw = [[None] * 2 for _ in range(2)]
            wtw2 = [[None] * 2 for _ in range(2)]
            for kk in range(2):
                for mm in range(2):
                    for tgt, srcd, nm in ((wtw, w1wt, "wtw"), (wtw2, w1w2t, "wtw2")):
                        t = const.tile([128, 128], F16, name=f"{nm}_{kk}{mm}")
                        nc.sync.dma_start(
                            out=t,
                            in_=bass.AP(
                                srcd.tensor, kk * 128 * H + mm * 128,
                                [[H, 128], [1, 128]],
                            ),
                        )
                        tgt[kk][mm] = t
            nid = const.tile([128, 128], F16, name="nid")
            nc.sync.dma_start(out=nid, in_=negid)

            bl = {}
            for l in (1, 2, 3):
                bl[l] = []
                for m in range(2):
                    t = new1(f"bl{l}_{m}")
                    nc.sync.dma_start(
                        out=t, in_=bass.AP(bs[l].tensor, 128 * m, [[1, 128], [1, 1]])
                    )
                    bl[l].append(t)

            # final-projection block-diagonal lhsT tiles: [128,3], col s = W4 half
            lt4 = [[None] * 2 for _ in range(3)]
            for s_idx in range(3):
                for kk in range(2):
                    if s_idx == 0:
                        t = const.tile([128, 3], F32R, name=f"lt4_{s_idx}{kk}")
                        nc.sync.dma_start(out=t, in_=lt4d[2 * s_idx + kk])
                    else:
                        t = const.tile([128, 3], F16, name=f"lt4_{s_idx}{kk}")
                        nc.sync.dma_start(out=t, in_=lt4h[2 * s_idx + kk])
                    lt4[s_idx][kk] = t

            # ---------- main loop over point chunks ----------
            y3f = sb.tile([3, NLOC], F32, name="y3f")
            for c in range(NCHUNK):
                x2c = sb.tile([2, CH], F32R, tag="x2c", bufs=2)
                nc.sync.dma_start(out=x2c, in_=x2[:, c * CH : (c + 1) * CH])

                Hv = [None] * 2
                H1 = [None] * 2
                H2 = [None] * 2

                # ----- layer 0: primal only; derivative tangents fold into
                # layer-1 weights (H1 <- dm, H2 <- a*d with scaled W1 copies) -----
                Dm0 = [None] * 2
                Ad0 = [None] * 2
                for m in range(2):
                    av = sb.tile([128, CH], F32R, tag=f"hv{m}", bufs=3, name="av")
                    ee = sb.tile([128, CH], F16, tag=f"ee{m}", bufs=2, name="ee")
                    dm = sb.tile([128, CH], F16, tag=f"dm{m}", bufs=2, name="dm")
                    ad = sb.tile([128, CH], F16, tag=f"ad{m}", bufs=2, name="ad")
                    for g in range(NG):
                        sl = slice(g * PG, (g + 1) * PG)
                        pz = ps.tile([128, PG], F32, tag="pz", bufs=8, name="pz")
                        for i in range(PG // 512):
                            o = g * PG + i * 512
                            nc.tensor.matmul(
                                pz[:, i * 512 : (i + 1) * 512],
                                w0ts[:, m * 128 : (m + 1) * 128],
                                x2c[:, o : o + 512],
                                start=True,
                                stop=True,
                            )
                        nc.scalar.activation(av[:, sl], pz, AF.Tanh, beta0[m])
                        nc.gpsimd.tensor_tensor(ee[:, sl], av[:, sl], av[:, sl], OP.mult)
                        nc.vector.tensor_scalar(
                            dm[:, sl], ee[:, sl], -1.0, 1.0, OP.mult, OP.add
                        )  # d = 1-e
                        nc.vector.tensor_tensor(
                            ad[:, sl], av[:, sl], dm[:, sl], OP.mult
                        )  # a*d
                    Hv[m], Dm0[m], Ad0[m] = av, dm, ad
                H1 = Dm0
                H2 = Ad0

                # ----- hidden layers 1..3 -----
                for l in (1, 2, 3):
                    nHv = [None] * 2
                    nEe = [None] * 2
                    nH1 = [None] * 2
                    nH2 = [None] * 2
                    St = [None] * 2
                    Tt = [None] * 2
                    # primal
                    for m in range(2):
                        pz = ps.tile([128, CH], F32, tag="pz", bufs=2)
                        for kk in range(2):
                            for i in range(PT):
                                nc.tensor.matmul(
                                    pz[:, i * 512 : (i + 1) * 512],
                                    wt[l][kk][m],
                                    Hv[kk][:, i * 512 : (i + 1) * 512],
                                    start=(kk == 0),
                                    stop=(kk == 1),
                                )
                        av = sb.tile([128, CH], F32R, tag=f"hv{m}", bufs=3)
                        nc.scalar.activation(av, pz, AF.Tanh, bl[l][m])
                        ee = sb.tile([128, CH], F16, tag=f"ee{m}", bufs=2)
                        nc.scalar.activation(ee, av, AF.Square)
                        nHv[m], nEe[m], nDm[m] = av, ee, dm
                    # first-derivative stream
                    for m in range(2):
                        pz1 = ps.tile([128, CH], F32, tag="pz", bufs=2)
                        for kk in range(2):
                            for i in range(PT):
                                nc.tensor.matmul(
                                    pz1[:, i * 512 : (i + 1) * 512],
                                    wt[l][kk][m],
                                    H1[kk][:, i * 512 : (i + 1) * 512],
                                    start=(kk == 0),
                                    stop=(kk == 1),
                                )
                        st = sb.tile([128, CH], F16, tag=f"st{m}", bufs=2)
                        nc.scalar.activation(st, pz1, AF.Square, 0.0, SQRT2)  # 2*z'^2
                        h1t = sb.tile([128, CH], F16, tag=f"h1{m}", bufs=3)
                        nc.vector.scalar_tensor_tensor(
                            h1t, nEe[m], 1.0, pz1, OP.subtract, OP.mult
                        )  # (e-1)*Z1
                        tt = sb.tile([128, CH], F16, tag=f"tt{m}", bufs=2)
                        nc.gpsimd.tensor_tensor(tt, nHv[m], st, OP.mult)  # a*s
                        nH1[m], St[m], Tt[m] = h1t, st, tt
                    # second-derivative stream
                    for m in range(2):
                        pz2 = ps.tile([128, CH], F32, tag="pz", bufs=2)
                        for kk in range(2):
                            for i in range(PT):
                                nc.tensor.matmul(
                                    pz2[:, i * 512 : (i + 1) * 512],
                                    wt[l][kk][m],
                                    H2[kk][:, i * 512 : (i + 1) * 512],
                                    start=(kk == 0),
                                    stop=(kk == 1),
                                )
                        qt = sb.tile([128, CH], F16, tag=f"qt{m}", bufs=2)
                        nc.vector.scalar_tensor_tensor(
                            qt, Tt[m], -s_in, pz2, OP.mult, OP.add
                        )  # sigma*(z''-t)
                        h2t = sb.tile([128, CH], F16, tag=f"h2{m}", bufs=3)
                        nc.vector.scalar_tensor_tensor(
                            h2t, nEe[m], 1.0, qt, OP.subtract, OP.mult
                        )  # (e-1)*q
                        nH2[m] = h2t
                    Hv, H1, H2 = nHv, nH1, nH2

                # ----- final projection: block-diag lhsT -> psum rows (y,y',y'') -----
                for i in range(PT):
                    py = ps.tile([3, 512], F32, tag="pz", bufs=8, name="py")
                    first = True
                    for s_idx, stream in enumerate((Hv, H1, H2)):
                        for kk in range(2):
                            nc.tensor.matmul(
                                py,
                                lt4[s_idx][kk],
                                stream[kk][:, i * 512 : (i + 1) * 512],
                                start=first,
                                stop=(s_idx == 2 and kk == 1),
                            )
                            first = False
                    nc.vector.tensor_copy(y3f[:, c * CH + i * 512 : c * CH + (i + 1) * 512], py)

            # ----- tail (once): reshape to [128, PPP] per stream, PDE algebra -----
            tp = sb.tile([128, 3 * PPP], F32, name="tp")
            for s_idx in range(3):
                nc.sync.dma_start(
                    out=tp[:, s_idx * PPP : (s_idx + 1) * PPP],
                    in_=y3f[s_idx : s_idx + 1, :],
                )
            yv = tp[:, 0:PPP]
            yt = tp[:, PPP : 2 * PPP]
            ytt = tp[:, 2 * PPP : 3 * PPP]
            oc = sb.tile([128, 3 * PPP], F32, name="oc")
            U = oc[:, 0:PPP]
            Fo = oc[:, PPP : 2 * PPP]
            Ft = oc[:, 2 * PPP : 3 * PPP]

            def tl(name):
                return sb.tile([128, PPP], F32, name=name)

            ut, utt, vv, v2, w1, q1, t1 = (
                tl("ut"), tl("utt"), tl("vv"), tl("v2"), tl("w1"), tl("q1"), tl("t1"),
            )
            nc.vector.tensor_scalar(U, yv, bc_ts, tmb, OP.mult, OP.add)
            nc.vector.tensor_scalar(ut, yt, sts, None, OP.mult)
            nc.vector.tensor_scalar(utt, ytt, sts, None, OP.mult)
            nc.vector.tensor_scalar(vv, U, C_t, None, OP.subtract)
            nc.vector.tensor_tensor(v2, vv, vv, OP.mult)
            nc.vector.scalar_tensor_tensor(w1, v2, c1, vv, OP.mult, OP.add)
            nc.vector.scalar_tensor_tensor(Fo, w1, nr, ut, OP.mult, OP.add)
            nc.vector.tensor_tensor(q1, vv, ut, OP.mult)
            nc.vector.scalar_tensor_tensor(t1, ut, nr, utt, OP.mult, OP.add)
            nc.vector.scalar_tensor_tensor(Ft, q1, mc3, t1, OP.mult, OP.add)
            for s_idx, srcap in enumerate((U, Fo, Ft)):
                nc.sync.dma_start(out=out[s_idx : s_idx + 1, :], in_=srcap)

    nc.compile()
    return nc


_STATE = {}


def _get_nc():
    if "nc" not in _STATE:
        _STATE["nc"] = _build()
    return _STATE["nc"]


def _make_lt4(w4):
    out = np.zeros((6, 128, 3), np.float32)
    for s_idx in range(3):
        for kk in range(2):
            out[2 * s_idx + kk, :, s_idx] = w4[0, kk * 128 : (kk + 1) * 128]
    return out


def _prep_in_maps(inputs):
    f = np.float32

    def arr(k):
        return np.ascontiguousarray(np.asarray(inputs[k], f))

    x = np.asarray(inputs["inputs"], f).reshape(N, 2)
    shared = {
        "w0t": np.ascontiguousarray(arr("W0").T),
        "w1t": np.ascontiguousarray(arr("W1").T),
        "w2t": np.ascontiguousarray(arr("W2").T),
        "w3t": np.ascontiguousarray(arr("W3").T),
        "lt4": _make_lt4(arr("W4").reshape(1, H)),
        "lt4h": _make_lt4(arr("W4").reshape(1, H)).astype(np.float16),
        "w1th": np.ascontiguousarray(arr("W1").T).astype(np.float16),
        "w1wt": np.ascontiguousarray(
            (arr("W1") * arr("W0")[:, 1][None, :]).T
        ).astype(np.float16),
        "w1w2t": np.ascontiguousarray(
            (arr("W1") * (-2.0 * arr("W0")[:, 1] ** 2)[None, :]).T
        ).astype(np.float16),
        "negid": (-np.eye(128)).astype(np.float16),
        "w2th": np.ascontiguousarray(arr("W2").T).astype(np.float16),
        "w3th": np.ascontiguousarray(arr("W3").T).astype(np.float16),
        "b0": arr("b0"),
        "b1": arr("b1"),
        "b2": arr("b2"),
        "b3": arr("b3"),
        "b4": arr("b4").reshape(1),
        "lgr": arr("log_growth_rate").reshape(1),
        "lcc": arr("log_carrying_capacity").reshape(1),
        "lil": arr("log_initial_loss").reshape(1),
        "in_mean": arr("in_mean"),
        "in_std": arr("in_std"),
        "tgt_mean": arr("tgt_mean"),
        "tgt_std": arr("tgt_std"),
    }
    in_maps = []
    for c in range(NCORES):
        m = dict(shared)
        m["x2"] = np.ascontiguousarray(x[c * NLOC : (c + 1) * NLOC].T)
        in_maps.append(m)
    return in_maps


def run(inputs, trace=False):
    nc = _get_nc()
    in_maps = _prep_in_maps(inputs)
    kw = {}
    if trace:
        kw["tmpdir"] = tempfile.mkdtemp(prefix="bassk_prof_")
    res = run_bass_kernel_spmd(
        nc, in_maps, core_ids=list(range(NCORES)), trace=trace, **kw
    )
    U = np.empty((N,), np.float32)
    F = np.empty((N,), np.float32)
    Ft = np.empty((N,), np.float32)
    for c in range(NCORES):
        o = res.results[c]["out"]
        U[c * NLOC : (c + 1) * NLOC] = o[0]
        F[c * NLOC : (c + 1) * NLOC] = o[1]
        Ft[c * NLOC : (c + 1) * NLOC] = o[2]
    shp = (B, S, 1)
    return (U.reshape(shp), F.reshape(shp), Ft.reshape(shp)), res


def kernel(**inputs):
    outs, _ = run(inputs, trace=False)
    return outs


# ---------------------------------------------------------------------------
# Dev-loop timing: persistent jitted executable (mirrors
# bass2jax.run_bass_via_pjrt's multi-core branch) so repeated executions
# reuse one compiled NEFF and can be timed back-to-back.
# ---------------------------------------------------------------------------
def _make_runner():
    if "runner" in _STATE:
        return _STATE["runner"]
    import jax
    from jax.experimental.shard_map import shard_map
    from jax.sharding import Mesh, PartitionSpec
    from concourse import bass2jax

    bass2jax.install_neuronx_cc_hook()
    nc = _get_nc()

    in_names, out_names, out_avals, zero_outs = [], [], [], []
    for alloc in nc.m.functions[0].allocations:
        if not isinstance(alloc, mybir.MemoryLocationSet):
            continue
        name = alloc.memorylocations[0].name
        if alloc.kind == "ExternalInput":
            if nc.partition_id_tensor is None or name != nc.partition_id_tensor.name:
                in_names.append(name)
        elif alloc.kind == "ExternalOutput":
            out_names.append(name)
            shape = tuple(alloc.tensor_shape)
            dtype = mybir.dt.np(alloc.dtype)
            out_avals.append(jax.core.ShapedArray(shape, dtype))
            zero_outs.append(np.zeros(shape, dtype))
    n_params = len(in_names)
    n_outs = len(out_avals)
    all_names = in_names + out_names
    if nc.partition_id_tensor is not None:
        all_names = all_names + [nc.partition_id_tensor.name]

    def _body(*args):
        operands = list(args)
        if nc.partition_id_tensor is not None:
            operands.append(bass2jax.partition_id_tensor())
        outs = bass2jax._bass_exec_p.bind(
            *operands,
            out_avals=tuple(out_avals),
            in_names=tuple(all_names),
            out_names=tuple(out_names),
            lowering_input_output_aliases=(),
            sim_require_finite=True,
            sim_require_nnan=True,
            nc=nc,
        )
        return tuple(outs)

    devices = jax.devices()[:NCORES]
    mesh = Mesh(np.asarray(devices), ("core",))
    donate = tuple(range(n_params, n_params + n_outs))
    sharded = jax.jit(
        shard_map(
            _body,
            mesh=mesh,
            in_specs=(PartitionSpec("core"),) * (n_params + n_outs),
            out_specs=(PartitionSpec("core"),) * n_outs,
            check_rep=False,
        ),
        donate_argnums=donate,
        keep_unused=True,
    )
    _STATE["runner"] = (sharded, in_names, out_names, out_avals, zero_outs)
    return _STATE["runner"]


def run_timed(inputs, iters=20):
    """Run via a persistent executable; return (outputs, per_iter_ns)."""
    import time as _time

    import jax

    sharded, in_names, out_names, out_avals, zero_outs = _make_runner()
    in_maps = _prep_in_maps(inputs)
    concat_in = [
        np.concatenate([np.asarray(in_maps[c][n]) for c in range(NCORES)], axis=0)
        for n in in_names
    ]
    dev_in = [jax.device_put(a) for a in concat_in]

    def zeros():
        return [
            np.zeros((NCORES * z.shape[0], *z.shape[1:]), z.dtype) for z in zero_outs
        ]

    # warmup (compiles on first call)
    outs = sharded(*dev_in, *zeros())
    jax.block_until_ready(outs)
    out_np = [np.asarray(o) for o in outs]

    zbufs = [zeros() for _ in range(iters)]
    t0 = _time.perf_counter()
    last = None
    for i in range(iters):
        last = sharded(*dev_in, *zbufs[i])
    jax.block_until_ready(last)
    t1 = _time.perf_counter()
    per_iter_ns = (t1 - t0) / iters * 1e9

    per_core = [
        {
            name: out_np[i].reshape(NCORES, *out_avals[i].shape)[c]
            for i, name in enumerate(out_names)
        }
        for c in range(NCORES)
    ]
    U = np.empty((N,), np.float32)
    F = np.empty((N,), np.float32)
    Ft = np.empty((N,), np.float32)
    for c in range(NCORES):
        o = per_core[c]["out"]
        U[c * NLOC : (c + 1) * NLOC] = o[0]
        F[c * NLOC : (c + 1) * NLOC] = o[1]
        Ft[c * NLOC : (c + 1) * NLOC] = o[2]
    shp = (B, S, 1)
    return (U.reshape(shp), F.reshape(shp), Ft.reshape(shp)), per_iter_ns
